# revision 21
# baseline (speedup 1.0000x reference)
"""MinGRU cell kernel for Trainium2 (8 NeuronCores, batch-parallel).

Reference computation (per batch b):
    k = x @ Wz.T + bz            # [S, H]
    u = x @ Wh.T + bh            # [S, H]
    z = sigmoid(k);  c = 1 - z = sigmoid(-k)
    g = where(u >= 0, u + 0.5, sigmoid(u)) = max(u + 0.5, sigmoid(u))
    h_t = c_t * h_{t-1} + z_t * g_t,   h_init = g(h_0)
The reference evaluates this scan in log-space; the linear-space recurrence is
a convex combination (c in (0,1), v >= 0) and is numerically tighter in fp32.
All device traffic and matmul inputs are fp16 (measured end-to-end max rel err
2.6e-3 vs the fp32 reference; fp16 matmul streams at the same 1 row/cycle PE
rate as f32r but halves DMA + SBUF and cheapens LDWEIGHTS).

Device layout: channels on partitions (768 = 6 x 128), time on the free axis.
  - TensorE: k/u projections, weights stationary ([d,h] tiles), x.T as rhs;
    dummy matmuls on memset tiles during the initial DMA window pre-warm the
    HAM clock gate so the real stream runs at 2.4 GHz from the first matmul.
  - ScalarE: the two sigmoids (biases via the per-partition bias operand).
  - VectorE: g and negv=(c-1)*g via fused scalar_tensor_tensor, and the scan
    via the native tensor_tensor_scan (state = c*state - negv) chained across
    chunks. Keeping the whole consumer chain on DVE matters: spreading it
    over GpSimd or a third ACT raises total engine activity enough to
    trigger a chip-wide ~18% downclock (measured on v3/v6a/v8 variants).
Each core processes one batch element. Host pre-packs x / weights into
partition-major layouts so every DMA line is >=1KB contiguous.
"""

import numpy as np

B, S, D, H = 8, 4096, 768, 768
TCH = 512                 # time chunk (= matmul moving free dim, PSUM bank)
NT = S // TCH             # 8 time chunks
KJ = D // 128             # 6 contraction sub-tiles
HJ = H // 128             # 6 channel tiles
NWARM = 3                 # chunk-major warm-up chunks (DMA ramp window)
NTAIL = 2                 # last unit consumer chain split into NTAIL pieces

_CACHE = {}


def _build_nc():
    import concourse.bacc as bacc
    import concourse.mybir as mybir
    import concourse.tile as tile

    fp32 = mybir.dt.float32
    f16 = mybir.dt.float16
    Act = mybir.ActivationFunctionType
    Alu = mybir.AluOpType

    nc = bacc.Bacc("TRN2", target_bir_lowering=False, debug=False)

    xt = nc.dram_tensor("xt", [128, NT, KJ, TCH], f16, kind="ExternalInput").ap()
    wzt = nc.dram_tensor("wzt", [128, HJ, KJ, 128], f16, kind="ExternalInput").ap()
    wht = nc.dram_tensor("wht", [128, HJ, KJ, 128], f16, kind="ExternalInput").ap()
    # per-partition scalars packed [128, 5, HJ]:
    # idx 0..4 = g(h0), -bz, +bz, +bh, bh+0.5; col j serves channel tile j
    scal = nc.dram_tensor("scal", [128, 5, HJ], fp32, kind="ExternalInput").ap()
    ht = nc.dram_tensor("ht", [HJ, 128, NT, TCH], f16, kind="ExternalOutput").ap()

    with tile.TileContext(nc) as tc:
        with (
            tc.tile_pool(name="consts", bufs=1) as consts,
            tc.tile_pool(name="temps", bufs=4) as temps,
            tc.tile_pool(name="hout", bufs=3) as hout,
            tc.tile_pool(name="psum", bufs=4, space="PSUM") as psum,
        ):
            wz_sb = consts.tile([128, HJ, KJ, 128], f16, tag="wz")
            wh_sb = consts.tile([128, HJ, KJ, 128], f16, tag="wh")
            sc_sb = consts.tile([128, 5, HJ], fp32, tag="scalars")

            # all of x stays resident: [128, NT, KJ, TCH] f16 = 48 KiB/part.
            x_all = consts.tile([128, NT, KJ, TCH], f16, tag="x")

            # DMA queues: issues on one queue serialize against the previous
            # transfer's completion (~1 in flight per queue). Critical-path
            # order: j0 weights + scal (Scalar queue, wakes earliest), chunk-0
            # x slices split across GpSimd/Sync, then the rest interleaved in
            # consumption order (j1-5 weights batched into two transfers).
            nc.scalar.dma_start(wh_sb[:, 0], wht[:, 0])
            nc.scalar.dma_start(wz_sb[:, 0], wzt[:, 0])
            nc.scalar.dma_start(sc_sb[:], scal)
            for a in (0, 1, 2):
                nc.gpsimd.dma_start(x_all[:, 0, a], xt[:, 0, a])
            for a in (3, 4, 5):
                nc.sync.dma_start(x_all[:, 0, a], xt[:, 0, a])
            # per-j weight blocks, wh on Scalar / wz on Sync so the two
            # streams land in parallel well before phase-1 consumes them
            for j in range(1, HJ):
                nc.scalar.dma_start(wh_sb[:, j], wht[:, j])
                nc.sync.dma_start(wz_sb[:, j], wzt[:, j])
            for i in range(1, NT):
                eng = nc.sync if i % 2 == 1 else nc.gpsimd
                eng.dma_start(x_all[:, i], xt[:, i])

            # PE warm-up: HAM un-throttles the PE clock (1.2 -> 2.4 GHz) only
            # after ~3.4us of sustained activity. Stream dummy matmuls on
            # memset tiles during the initial DMA window so the real matmuls
            # start warm.
            dum_w = temps.tile([128, 128], f16, tag="dumw")
            dum_x = temps.tile([128, TCH], f16, tag="dumx")
            nc.vector.memset(dum_w[:], 0.0)
            nc.vector.memset(dum_x[:], 0.0)
            for _ in range(9):
                pwarm = psum.tile([128, TCH], fp32, tag="pu")
                nc.tensor.matmul(pwarm[:], dum_w[:], dum_x[:],
                                 start=True, stop=True)

            # preload the sigmoid table set before the first real ACTIVATE
            warm_sb = temps.tile([128, HJ], fp32, tag="warm")
            nc.scalar.activation(warm_sb[:], sc_sb[:, 0], Act.Sigmoid)
            # phase-1 scan carries (one column per channel tile)
            carry = consts.tile([128, HJ], fp32, tag="carry")

            # phase 1 chunk-major (DMA warm-up window), phase 2 channel-major
            # (single-pipeline drain at the kernel tail)
            order = [(i, j) for i in range(NWARM) for j in range(HJ)] + \
                    [(i, j) for j in range(HJ) for i in range(NWARM, NT)]
            h_prev = [None] * HJ
            for i, j in order:
                last_unit = (i == NT - 1 and j == HJ - 1)
                pu = psum.tile([128, TCH], fp32, tag="pu")
                pk = psum.tile([128, TCH], fp32, tag="pk")
                # u first: sg -> stt(g) unblocks VectorE earliest
                for a in range(KJ):
                    nc.tensor.matmul(pu[:], wh_sb[:, j, a], x_all[:, i, a],
                                     start=(a == 0), stop=(a == KJ - 1))
                for a in range(KJ):
                    nc.tensor.matmul(pk[:], wz_sb[:, j, a], x_all[:, i, a],
                                     start=(a == 0), stop=(a == KJ - 1))

                nsub = NTAIL if last_unit else 1
                sub = TCH // nsub
                for si in range(nsub):
                    fsl = slice(si * sub, (si + 1) * sub)
                    c_sb = temps.tile([128, sub], f16, tag=f"c{si}")
                    sg_sb = temps.tile([128, sub], f16, tag=f"sg{si}")
                    g_sb = temps.tile([128, sub], f16, tag=f"g{si}")
                    v_sb = temps.tile([128, sub], f16, tag=f"v{si}")
                    nc.scalar.activation(sg_sb[:], pu[:, fsl], Act.Sigmoid,
                                         bias=sc_sb[:, 3, j:j + 1])
                    nc.scalar.activation(c_sb[:], pk[:, fsl], Act.Sigmoid,
                                         bias=sc_sb[:, 1, j:j + 1], scale=-1.0)
                    nc.vector.scalar_tensor_tensor(g_sb[:], pu[:, fsl],
                                                   sc_sb[:, 4, j:j + 1], sg_sb[:],
                                                   op0=Alu.add, op1=Alu.max)
                    # negv = (c-1)*g = -(1-c)*g; scan: c*h - negv = c*h+(1-c)*g
                    nc.vector.scalar_tensor_tensor(v_sb[:], c_sb[:], -1.0,
                                                   g_sb[:],
                                                   op0=Alu.add, op1=Alu.mult)
                    h_sb = hout.tile([128, sub], f16, tag=f"h{si}")
                    if si > 0:
                        init = prev_sub[:, sub - 1:sub]
                    elif i == 0:
                        init = sc_sb[:, 0, j:j + 1]
                    elif i <= NWARM:
                        init = carry[:, j:j + 1]
                    else:
                        init = h_prev[j][:, h_prev[j].shape[-1] - 1:]
                    nc.vector.tensor_tensor_scan(h_sb[:], c_sb[:], v_sb[:],
                                                 init,
                                                 op0=Alu.mult, op1=Alu.subtract)
                    if i < NWARM and si == nsub - 1:
                        nc.scalar.copy(carry[:, j:j + 1], h_sb[:, sub - 1:sub])
                    prev_sub = h_sb
                    nc.sync.dma_start(ht[j, :, i, fsl], h_sb[:])
                h_prev[j] = prev_sub
    nc.compile()
    return nc


def _get_nc():
    if "nc" not in _CACHE:
        _CACHE["nc"] = _build_nc()
    return _CACHE["nc"]


def _sigmoid(x):
    return 1.0 / (1.0 + np.exp(-x))


def _host_inputs(x, h_0, Wz, bz, Wh, bh):
    """Build the per-core input maps (host-side layout only)."""
    x = np.asarray(x, dtype=np.float32)
    h_0 = np.asarray(h_0, dtype=np.float32)
    Wz = np.asarray(Wz, dtype=np.float32)
    Wh = np.asarray(Wh, dtype=np.float32)
    bz = np.asarray(bz, dtype=np.float32)
    bh = np.asarray(bh, dtype=np.float32)
    b, s, d = x.shape
    h = Wz.shape[0]
    # x[b, i*TCH+t, a*128+p] -> xt[b, p, i, a, t]
    xt = np.ascontiguousarray(
        x.astype(np.float16).reshape(b, NT, TCH, KJ, 128).transpose(0, 4, 1, 3, 2))
    # W[j*128+q, a*128+p] -> wt[p, j, a, q]
    def wpack(W):
        return np.ascontiguousarray(
            W.astype(np.float16).reshape(HJ, 128, KJ, 128).transpose(3, 0, 2, 1))
    wzt = wpack(Wz)
    wht = wpack(Wh)
    h0 = h_0.reshape(b, h)
    h0g = np.maximum(h0 + 0.5, _sigmoid(h0)).astype(np.float32)   # g(h_0)

    def cols(vec):  # [H] -> [128, HJ] with arr[p, j] = vec[j*128+p]
        return np.ascontiguousarray(vec.reshape(h // 128, 128).T.astype(np.float32))

    in_maps = []
    for bi in range(b):
        scal = np.stack([cols(h0g[bi]), cols(-bz), cols(bz),
                         cols(bh), cols(bh + 0.5)], axis=1)  # [128, 5, HJ]
        in_maps.append({
            "xt": xt[bi],
            "wzt": wzt,
            "wht": wht,
            "scal": np.ascontiguousarray(scal),
        })
    return in_maps


def run_device(x, h_0, Wz, bz, Wh, bh, trace=False, **trace_kwargs):
    """Run on the 8 NeuronCores; returns (out [B,S,H], BassKernelResults)."""
    from concourse.bass_utils import run_bass_kernel_spmd

    in_maps = _host_inputs(x, h_0, Wz, bz, Wh, bh)
    nc = _get_nc()
    res = run_bass_kernel_spmd(nc, in_maps, core_ids=list(range(len(in_maps))),
                               trace=trace, **trace_kwargs)
    # ht [HJ, 128, NT, TCH] f16 -> out[b, i*TCH+t, j*128+p]
    out = np.stack([
        r["ht"].transpose(2, 3, 0, 1).reshape(S, H).astype(np.float32)
        for r in res.results])
    return out, res


def kernel(x, h_0, Wz, bz, Wh, bh):
    out, _ = run_device(x, h_0, Wz, bz, Wh, bh)
    return out
